# revision 12
# baseline (speedup 1.0000x reference)
"""Trainium2 Bass kernel for nn_CNN_88098369175781.

Model: x[1,1,18,T=262144] -> wavA=x[...,0,:], eeg=x[...,1:17,:], wavB=x[...,17,:]
  wav streams: proj(1->16, pointwise) -> diagonal sinc filter bank (15 taps,
  pad 7) -> conv(16->10, 9 taps) + bias -> relu -> global max-pool.
  eeg stream:  conv(16->10, 9 taps) + bias -> relu -> global max-pool.
  concat -> sigmoid FC(30->30) -> sigmoid FC(30->2).

Device decomposition:
  * Wav streams fuse to ONE 1->10ch 23-tap conv (precomposed host-side).
  * Bias/relu commute past the global max; device computes convs + maxima.
  * eeg conv via B=12 polyphase: out[o, s+12m+dt], M=120=(o,dt), three
    accumulating passes: A (s'=r in [0,8), K=128), C (s'=12+r from the
    m+1 column, K=128), B (s'=8+r' in [8,12), K=64).
  * wav via 12-phase polyphase (K=36, M=120); the A and B streams run as
    row-tiled concurrent matmuls (tile_position (0,0) / (64,0)).
  * PSUM evacuation: ACT casts some banks to fp16 SBUF; GpSimd folds the
    stream tails into those casts in-place (tensor_tensor max); DVE
    tensor_tensor_reduce consumes a fresh PSUM bank and a cast tile per
    pass, writing per-row maxima into the output tile.
  * 8 cores split the time axis (overlapping chunks; overlap free for max).
  * Host combines per-core maxima and runs the tiny FC head.
"""

import os
import numpy as np

T = 262144
NOUT = T - 8            # 262136 valid conv output positions
NCORES = 8
KLEN = 15
SIGMA = 0.005

B12 = 12
NCOL = 2731             # phase columns per core (12*2731 = 32772 outputs)
TC = 12 * NCOL          # outputs per core

_NC_CACHE = {}
LAST_RESULT = None      # BassKernelResults of the most recent device run


# --------------------------------------------------------------------------
# host-side weight precompute
# --------------------------------------------------------------------------

def _sinc_rows(mu):
    """Diagonal rows of the reference's sinc_kernel: [16, 15] float64."""
    k = np.linspace(-1.0, 1.0, KLEN)
    kk = (k[None, :] - np.asarray(mu, np.float64)[:, None]) / SIGMA
    nos = np.sum(np.abs(kk) < 1e-5, axis=1)
    kk = np.where((nos >= 0.5)[:, None], kk - 5e-5, kk)
    return np.sin(np.pi * kk) / (np.pi * kk)


def _composite_wav_weights(mu, proj_w, conv_w_i):
    """Fused 1->10ch 23-tap kernel E[o, s] (float64)."""
    krn = _sinc_rows(mu)                                  # [16,15]
    a = np.asarray(proj_w, np.float64)[:, 0, 0]           # [16]
    W = np.asarray(conv_w_i, np.float64)                  # [10,16,9]
    E = np.zeros((10, 23))
    for j in range(9):
        E[:, j:j + 15] += np.einsum('oc,cm->om', W[:, :, j] * a[None, :], krn)
    return E


def _eeg_lhsT12(W1):
    """B=12 eeg weights: (W_A [128,120], W_C [128,120], W_B [64,120]).

    cols (o*12+dt); A rows (c*8+r): W1[o,c,r-dt]; C rows (c*8+r):
    W1[o,c,12+r-dt]; B rows (c*4+r'): W1[o,c,8+r'-dt]."""
    W1 = np.asarray(W1, np.float64)         # [10,16,9]
    WA = np.zeros((128, 120))
    WC = np.zeros((128, 120))
    WB = np.zeros((64, 120))
    for o in range(10):
        for dt in range(12):
            col = o * 12 + dt
            for c in range(16):
                for r in range(8):
                    j = r - dt
                    if 0 <= j < 9:
                        WA[c * 8 + r, col] = W1[o, c, j]
                    j = 12 + r - dt
                    if 0 <= j < 9:
                        WC[c * 8 + r, col] = W1[o, c, j]
                for rp in range(4):
                    j = 8 + rp - dt
                    if 0 <= j < 9:
                        WB[c * 4 + rp, col] = W1[o, c, j]
    return (WA.astype(np.float32), WC.astype(np.float32),
            WB.astype(np.float32))


def _wav_lhsT(E):
    """[36, 120]: row v*3+q, col o*12+dt, val E[o, 12q+v-dt]."""
    out = np.zeros((36, 120))
    v, q, o, dt = np.meshgrid(np.arange(12), np.arange(3), np.arange(10),
                              np.arange(12), indexing='ij')
    s = 12 * q + v - dt
    valid = (s >= 0) & (s < 23)
    out[(v * 3 + q)[valid], (o * 12 + dt)[valid]] = E[o[valid], np.clip(s[valid], 0, 22)]
    return out.astype(np.float32)


# --------------------------------------------------------------------------
# host-side per-core input slicing
# --------------------------------------------------------------------------

def _core_start(k):
    return min(k * 32767, NOUT - TC)


def _eeg_phases12(eegp, k):
    """eegp: [16, T+pad] fp16. Returns (plo [128, 2732], phi [64, 2731])."""
    s = _core_start(k)
    v = eegp[:, s:s + 12 * (NCOL + 1)]                    # [16, 32784]
    p = v.reshape(16, NCOL + 1, 12)                       # [16, 2732, 12]
    plo = p[:, :, 0:8].transpose(0, 2, 1).reshape(128, NCOL + 1)
    phi = p[:, 0:NCOL, 8:12].transpose(0, 2, 1).reshape(64, NCOL)
    return np.ascontiguousarray(plo), np.ascontiguousarray(phi)


def _wav_phases(w_pad, k):
    """[36, 2731]: row v*3+q, col n = w_pad[s + 12(n+q) + v]."""
    s = _core_start(k)
    sl = w_pad[s:s + 12 * (NCOL + 2)]
    y = sl.reshape(NCOL + 2, 12).T                        # y[v,m] = sl[12m+v]
    out = np.empty((36, NCOL), dtype=w_pad.dtype)
    for q in range(3):
        out[q::3, :] = y[:, q:q + NCOL]
    return np.ascontiguousarray(out)


# --------------------------------------------------------------------------
# bass kernel
# --------------------------------------------------------------------------

def _build_nc():
    import concourse.bacc as bacc
    import concourse.tile as tile
    import concourse.mybir as mybir

    f32 = mybir.dt.float32
    f16 = mybir.dt.float16
    Max = mybir.AluOpType.max
    X = mybir.AxisListType.X
    Copy = mybir.ActivationFunctionType.Copy

    no_ttr = bool(os.environ.get("KV_NO_TTR"))
    no_tilepos = bool(os.environ.get("KV_NO_TILEPOS"))
    nc = bacc.Bacc("TRN2", target_bir_lowering=False, debug=False,
                   num_devices=NCORES)

    # DRAM I/O
    wts = nc.dram_tensor("wts", [128, 480], f16, kind="ExternalInput")
    plo = nc.dram_tensor("plo", [128, NCOL + 1], f16, kind="ExternalInput")
    phi = nc.dram_tensor("phi", [64, NCOL], f16, kind="ExternalInput")
    wavA = nc.dram_tensor("wavA", [36, NCOL], f16, kind="ExternalInput")
    wavB = nc.dram_tensor("wavB", [36, NCOL], f16, kind="ExternalInput")
    out = nc.dram_tensor("out", [128, 8], f16, kind="ExternalOutput")

    NINIT = -60000.0
    N_WARM = 3

    with tile.TileContext(nc) as tc:
        with (
            tc.tile_pool(name="sb", bufs=1) as sb,
            tc.tile_pool(name="ps", bufs=4, space="PSUM") as psp,
        ):
            # ---- SBUF tiles
            scr = sb.tile([128, 512], f16, tag="scr")
            wtsT = sb.tile([128, 480], f16, tag="wtsT")
            ploT0 = sb.tile([128, 1025], f16, tag="ploT0")
            ploT1 = sb.tile([128, 1025], f16, tag="ploT1")
            ploT2 = sb.tile([128, 684], f16, tag="ploT2")
            phiT01 = sb.tile([64, 2048], f16, tag="phiT01")
            phiT2 = sb.tile([64, 683], f16, tag="phiT2")
            wavTA = sb.tile([36, NCOL], f16, tag="wavTA")
            wavTB = sb.tile([128, NCOL], f16, tag="wavTB")  # rows 64..99 used
            cE0 = sb.tile([120, 1024], f16, tag="cE0")
            cE1 = sb.tile([120, 1024], f16, tag="cE1")
            cE2 = sb.tile([120, 683], f16, tag="cE2")
            fE = sb.tile([120, 1024], f16, tag="fE")
            out32 = sb.tile([128, 8], f16, tag="out32")

            # ---- warmup scratch init first so PE can start immediately
            nc.gpsimd.memset(scr[:], 0.0)
            nc.gpsimd.memset(out32[:], NINIT)

            # ---- input DMAs: split by need-time across SP / ACT / SWDGE
            # rings (~110 GB/s each, FIFO within a ring).
            nc.sync.dma_start(wtsT[:], wts[:])
            nc.sync.dma_start(ploT0[:, 0:513], plo[:, 0:513])
            nc.sync.dma_start(ploT1[:, 0:513], plo[:, 1024:1537])
            nc.sync.dma_start(ploT2[:], plo[:, 2048:2732])
            nc.scalar.dma_start(phiT01[:], phi[:, 0:2048])
            nc.scalar.dma_start(ploT0[:, 513:1025], plo[:, 513:1025])
            nc.scalar.dma_start(ploT1[:, 513:1025], plo[:, 1537:2049])
            nc.scalar.dma_start(phiT2[:], phi[:, 2048:2731])
            nc.gpsimd.dma_start(wavTA[:], wavA[:])
            nc.gpsimd.dma_start(wavTB[64:100, :], wavB[:])

            # ---- PE warmup on scratch (keeps HAM busy while DMAs land)
            psD = psp.tile([120, 1024], f32, tag="u", name="psD")
            for _ in range(N_WARM):
                nc.tensor.matmul(psD[0:80, 0:512], scr[:, 0:80], scr[:],
                                 start=True, stop=True)

            wA = wtsT[:, 0:120]          # eeg pass A (K=128)
            wC = wtsT[:, 120:240]        # eeg pass C (K=128)
            wB = wtsT[0:64, 240:360]     # eeg pass B (K=64)
            wWA = wtsT[0:36, 360:480]    # wav A lhsT
            wWB = wtsT[64:100, 360:480]  # wav B lhsT

            def eeg_unit(ps, pl, lo, n, ph, phlo):
                for j in range(0, n, 512):
                    w = min(512, n - j)
                    nc.tensor.matmul(ps[:, j:j + w], wA,
                                     pl[:, lo + j:lo + j + w],
                                     start=True, stop=False)
                    nc.tensor.matmul(ps[:, j:j + w], wC,
                                     pl[:, lo + j + 1:lo + j + 1 + w],
                                     start=False, stop=False)
                    nc.tensor.matmul(ps[:, j:j + w], wB,
                                     ph[:, phlo + j:phlo + j + w],
                                     start=False, stop=True)

            def wav_pair(psA, psB, c0, n):
                tpA = None if no_tilepos else (0, 0)
                tpB = None if no_tilepos else (64, 0)
                for j in range(0, n, 512):
                    w = min(512, n - j)
                    nc.tensor.matmul(psA[:, j:j + w], wWA,
                                     wavTA[:, c0 + j:c0 + j + w],
                                     start=True, stop=True,
                                     tile_position=tpA)
                    nc.tensor.matmul(psB[:, j:j + w], wWB,
                                     wavTB[64:100, c0 + j:c0 + j + w],
                                     start=True, stop=True,
                                     tile_position=tpB)

            # ---- matmul schedule (PSUM pool rotates 4x [120,1024] tiles)
            psE0 = psp.tile([120, 1024], f32, tag="u", name="psE0")
            eeg_unit(psE0, ploT0, 0, 1024, phiT01, 0)
            nc.scalar.activation(cE0[:], psE0[:], Copy)

            psE1 = psp.tile([120, 1024], f32, tag="u", name="psE1")
            eeg_unit(psE1, ploT1, 0, 1024, phiT01, 1024)
            nc.scalar.activation(cE1[:], psE1[:], Copy)

            psA0 = psp.tile([120, 1024], f32, tag="u", name="psA0")
            psB0 = psp.tile([120, 1024], f32, tag="u", name="psB0")
            wav_pair(psA0, psB0, 0, 1024)
            nc.vector.tensor_tensor(fE[:], cE0[:], cE1[:], Max)
            nc.vector.tensor_reduce(out32[0:120, 1:2], psA0[:], X, Max)
            nc.vector.tensor_reduce(out32[0:120, 2:3], psB0[:], X, Max)

            psE2 = psp.tile([120, 1024], f32, tag="u", name="psE2")
            eeg_unit(psE2, ploT2, 0, 683, phiT2, 0)
            nc.scalar.activation(cE2[:], psE2[:, 0:683], Copy)

            psA1 = psp.tile([120, 1024], f32, tag="u", name="psA1")
            psB1 = psp.tile([120, 1024], f32, tag="u", name="psB1")
            wav_pair(psA1, psB1, 1024, 1024)
            nc.vector.tensor_tensor(fE[:, 0:683], fE[:, 0:683], cE2[:], Max)
            nc.vector.tensor_reduce(out32[0:120, 0:1], fE[:], X, Max)
            nc.vector.tensor_reduce(out32[0:120, 3:4], psA1[:], X, Max)
            nc.vector.tensor_reduce(out32[0:120, 4:5], psB1[:], X, Max)

            psA2 = psp.tile([120, 1024], f32, tag="u", name="psA2")
            psB2 = psp.tile([120, 1024], f32, tag="u", name="psB2")
            wav_pair(psA2, psB2, 2048, 683)
            nc.vector.tensor_reduce(out32[0:120, 5:6], psA2[:, 0:683], X, Max)
            nc.vector.tensor_reduce(out32[0:120, 6:7], psB2[:, 0:683], X, Max)

            nc.sync.dma_start(out[:], out32[:])

    nc.compile()
    return nc


def _get_nc():
    if "nc" not in _NC_CACHE:
        _NC_CACHE["nc"] = _build_nc()
    return _NC_CACHE["nc"]


# --------------------------------------------------------------------------
# entry point
# --------------------------------------------------------------------------

def _prepare_in_maps(x, mu, projA_w, projB_w, conv_w):
    x = np.asarray(x, np.float32)
    eegp = np.concatenate([x[0, 0, 1:17, :], np.zeros((16, 64), np.float32)],
                          axis=1).astype(np.float16)
    zt = np.zeros(64, np.float32)
    w_padA = np.concatenate([np.zeros(7, np.float32), x[0, 0, 0, :], zt]
                            ).astype(np.float16)
    w_padB = np.concatenate([np.zeros(7, np.float32), x[0, 0, 17, :], zt]
                            ).astype(np.float16)

    conv_w = np.asarray(conv_w)
    E_A = _composite_wav_weights(mu, projA_w, conv_w[0])
    E_B = _composite_wav_weights(mu, projB_w, conv_w[2])
    WA, WC, WB = _eeg_lhsT12(conv_w[1])
    wts_np = np.zeros((128, 480), np.float16)
    wts_np[:, 0:120] = WA
    wts_np[:, 120:240] = WC
    wts_np[0:64, 240:360] = WB
    wts_np[0:36, 360:480] = _wav_lhsT(E_A)
    wts_np[64:100, 360:480] = _wav_lhsT(E_B)

    in_maps = []
    for k in range(NCORES):
        plo_k, phi_k = _eeg_phases12(eegp, k)
        in_maps.append({
            "wts": wts_np,
            "plo": plo_k,
            "phi": phi_k,
            "wavA": _wav_phases(w_padA, k),
            "wavB": _wav_phases(w_padB, k),
        })
    return in_maps


def _head(percore, conv_b, fc1_w, fc1_b, fc2_w, fc2_b):
    m = percore.max(axis=0).astype(np.float64)            # [360]
    eeg_o = m[0:120].reshape(10, 12).max(axis=1)
    wavA_o = m[120:240].reshape(10, 12).max(axis=1)
    wavB_o = m[240:360].reshape(10, 12).max(axis=1)
    conv_b = np.asarray(conv_b, np.float64)
    f = np.concatenate([np.maximum(wavA_o + conv_b[0], 0.0),
                        np.maximum(eeg_o + conv_b[1], 0.0),
                        np.maximum(wavB_o + conv_b[2], 0.0)])
    h = 1.0 / (1.0 + np.exp(-(f @ np.asarray(fc1_w, np.float64).T
                              + np.asarray(fc1_b, np.float64))))
    o = 1.0 / (1.0 + np.exp(-(h @ np.asarray(fc2_w, np.float64).T
                              + np.asarray(fc2_b, np.float64))))
    return o[None, :].astype(np.float32)


def _percore_from_out(arr):
    """Device 'out' [128,8] fp16 -> flat [360] (eeg 120, wavA 120, wavB 120).

    cols: 0=eeg; 1,3,5=wavA; 2,4,6=wavB."""
    arr = np.asarray(arr, np.float32)
    return np.concatenate([arr[0:120, 0],
                           arr[0:120, [1, 3, 5]].max(axis=1),
                           arr[0:120, [2, 4, 6]].max(axis=1)])


def kernel(x, mu, projA_w, projB_w, conv_w, conv_b, fc1_w, fc1_b, fc2_w, fc2_b):
    global LAST_RESULT
    in_maps = _prepare_in_maps(x, mu, projA_w, projB_w, conv_w)
    nc = _get_nc()

    if os.environ.get("KERNEL_USE_SIM"):
        from concourse.bass_interp import CoreSim
        percore = np.zeros((NCORES, 360), np.float32)
        for k in range(NCORES):
            sim = CoreSim(nc)
            for name, arr in in_maps[k].items():
                sim.tensor(name)[:] = arr
            sim.simulate()
            percore[k] = _percore_from_out(sim.tensor("out"))
    else:
        from concourse.bass_utils import run_bass_kernel_spmd
        trace = bool(os.environ.get("KERNEL_TRACE"))
        res = run_bass_kernel_spmd(nc, in_maps, list(range(NCORES)),
                                   trace=trace)
        LAST_RESULT = res
        percore = np.stack([_percore_from_out(res.results[k]["out"])
                            for k in range(NCORES)])

    return _head(percore, conv_b, fc1_w, fc1_b, fc2_w, fc2_b)


# --------------------------------------------------------------------------
# numpy self-check of the host-side math (no hardware needed)
# --------------------------------------------------------------------------

def _selfcheck():
    rng = np.random.default_rng(0)
    Tm = 12 * (NCOL + 2) + 64
    eeg = rng.standard_normal((16, T)).astype(np.float32)
    W1 = (rng.standard_normal((10, 16, 9)) * 0.1).astype(np.float32)

    # reference conv for a window
    k = 3
    s = _core_start(k)
    ref = np.zeros((10, TC))
    for j in range(9):
        ref += np.einsum('oc,ct->ot', W1[:, :, j],
                         eeg[:, s + j:s + j + TC])

    eegp = np.concatenate([eeg, np.zeros((16, 64), np.float32)], axis=1)
    plo_k, phi_k = _eeg_phases12(eegp, k)
    WA, WC, WB = _eeg_lhsT12(W1)

    # emulate the three passes
    got = np.zeros((120, NCOL))
    got += WA.T @ plo_k[:, 0:NCOL]
    got += WC.T @ plo_k[:, 1:NCOL + 1]
    got += WB.T @ phi_k[:, 0:NCOL]
    got_ot = got.reshape(10, 12, NCOL).transpose(0, 2, 1).reshape(10, TC)
    err = np.abs(got_ot - ref).max()
    print("eeg B=12 max err:", err)
    assert err < 2e-2, err

    # wav path check
    wav = rng.standard_normal(T).astype(np.float32)
    E = rng.standard_normal((10, 23)) * 0.1
    w_pad = np.concatenate([np.zeros(7, np.float32), wav,
                            np.zeros(64, np.float32)]).astype(np.float16)
    ph = _wav_phases(w_pad, k)
    L = _wav_lhsT(E)
    gotw = (L.T @ ph.astype(np.float64)).reshape(10, 12, NCOL)
    gotw = gotw.transpose(0, 2, 1).reshape(10, TC)
    refw = np.zeros((10, TC))
    wp = np.concatenate([np.zeros(7), wav.astype(np.float64)])
    for j in range(23):
        refw += np.outer(E[:, j], wp[s + j:s + j + TC])
    errw = np.abs(gotw - refw).max()
    print("wav max err:", errw)
    assert errw < 2e-2, errw
    print("selfcheck OK")


if __name__ == "__main__":
    _selfcheck()


# revision 13
# speedup vs baseline: 1.0126x; 1.0126x over previous
"""Trainium2 Bass kernel for nn_CNN_88098369175781.

Model: x[1,1,18,T=262144] -> wavA=x[...,0,:], eeg=x[...,1:17,:], wavB=x[...,17,:]
  wav streams: proj(1->16, pointwise) -> diagonal sinc filter bank (15 taps,
  pad 7) -> conv(16->10, 9 taps) + bias -> relu -> global max-pool.
  eeg stream:  conv(16->10, 9 taps) + bias -> relu -> global max-pool.
  concat -> sigmoid FC(30->30) -> sigmoid FC(30->2).

Device decomposition:
  * Wav streams fuse to ONE 1->10ch 23-tap conv (precomposed host-side).
  * Bias/relu commute past the global max; device computes convs + maxima.
  * eeg conv via B=12 polyphase: out[o, s+12m+dt], M=120=(o,dt), three
    accumulating passes: A (s'=r in [0,8), K=128), C (s'=12+r from the
    m+1 column, K=128), B (s'=8+r' in [8,12), K=64).
  * wav via 12-phase polyphase (K=36, M=120); the A and B streams run as
    row-tiled concurrent matmuls (tile_position (0,0) / (64,0)).
  * PSUM evacuation: ACT casts some banks to fp16 SBUF; GpSimd folds the
    stream tails into those casts in-place (tensor_tensor max); DVE
    tensor_tensor_reduce consumes a fresh PSUM bank and a cast tile per
    pass, writing per-row maxima into the output tile.
  * 8 cores split the time axis (overlapping chunks; overlap free for max).
  * Host combines per-core maxima and runs the tiny FC head.
"""

import os
import numpy as np

T = 262144
NOUT = T - 8            # 262136 valid conv output positions
NCORES = 8
KLEN = 15
SIGMA = 0.005

B12 = 12
NCOL = 2731             # phase columns per core (12*2731 = 32772 outputs)
TC = 12 * NCOL          # outputs per core

_NC_CACHE = {}
LAST_RESULT = None      # BassKernelResults of the most recent device run


# --------------------------------------------------------------------------
# host-side weight precompute
# --------------------------------------------------------------------------

def _sinc_rows(mu):
    """Diagonal rows of the reference's sinc_kernel: [16, 15] float64."""
    k = np.linspace(-1.0, 1.0, KLEN)
    kk = (k[None, :] - np.asarray(mu, np.float64)[:, None]) / SIGMA
    nos = np.sum(np.abs(kk) < 1e-5, axis=1)
    kk = np.where((nos >= 0.5)[:, None], kk - 5e-5, kk)
    return np.sin(np.pi * kk) / (np.pi * kk)


def _composite_wav_weights(mu, proj_w, conv_w_i):
    """Fused 1->10ch 23-tap kernel E[o, s] (float64)."""
    krn = _sinc_rows(mu)                                  # [16,15]
    a = np.asarray(proj_w, np.float64)[:, 0, 0]           # [16]
    W = np.asarray(conv_w_i, np.float64)                  # [10,16,9]
    E = np.zeros((10, 23))
    for j in range(9):
        E[:, j:j + 15] += np.einsum('oc,cm->om', W[:, :, j] * a[None, :], krn)
    return E


def _eeg_lhsT12(W1):
    """B=12 eeg weights: (W_A [128,120], W_C [128,120], W_B [64,120]).

    cols (o*12+dt); A rows (c*8+r): W1[o,c,r-dt]; C rows (c*8+r):
    W1[o,c,12+r-dt]; B rows (c*4+r'): W1[o,c,8+r'-dt]."""
    W1 = np.asarray(W1, np.float64)         # [10,16,9]
    WA = np.zeros((128, 120))
    WC = np.zeros((128, 120))
    WB = np.zeros((64, 120))
    for o in range(10):
        for dt in range(12):
            col = o * 12 + dt
            for c in range(16):
                for r in range(8):
                    j = r - dt
                    if 0 <= j < 9:
                        WA[c * 8 + r, col] = W1[o, c, j]
                    j = 12 + r - dt
                    if 0 <= j < 9:
                        WC[c * 8 + r, col] = W1[o, c, j]
                for rp in range(4):
                    j = 8 + rp - dt
                    if 0 <= j < 9:
                        WB[c * 4 + rp, col] = W1[o, c, j]
    return (WA.astype(np.float32), WC.astype(np.float32),
            WB.astype(np.float32))


def _wav_lhsT(E):
    """[36, 120]: row v*3+q, col o*12+dt, val E[o, 12q+v-dt]."""
    out = np.zeros((36, 120))
    v, q, o, dt = np.meshgrid(np.arange(12), np.arange(3), np.arange(10),
                              np.arange(12), indexing='ij')
    s = 12 * q + v - dt
    valid = (s >= 0) & (s < 23)
    out[(v * 3 + q)[valid], (o * 12 + dt)[valid]] = E[o[valid], np.clip(s[valid], 0, 22)]
    return out.astype(np.float32)


# --------------------------------------------------------------------------
# host-side per-core input slicing
# --------------------------------------------------------------------------

def _core_start(k):
    return min(k * 32767, NOUT - TC)


def _eeg_phases12(eegp, k):
    """eegp: [16, T+pad] fp16. Returns (plo [128, 2732], phi [64, 2731])."""
    s = _core_start(k)
    v = eegp[:, s:s + 12 * (NCOL + 1)]                    # [16, 32784]
    p = v.reshape(16, NCOL + 1, 12)                       # [16, 2732, 12]
    plo = p[:, :, 0:8].transpose(0, 2, 1).reshape(128, NCOL + 1)
    phi = p[:, 0:NCOL, 8:12].transpose(0, 2, 1).reshape(64, NCOL)
    return np.ascontiguousarray(plo), np.ascontiguousarray(phi)


def _wav_phases(w_pad, k):
    """[36, 2731]: row v*3+q, col n = w_pad[s + 12(n+q) + v]."""
    s = _core_start(k)
    sl = w_pad[s:s + 12 * (NCOL + 2)]
    y = sl.reshape(NCOL + 2, 12).T                        # y[v,m] = sl[12m+v]
    out = np.empty((36, NCOL), dtype=w_pad.dtype)
    for q in range(3):
        out[q::3, :] = y[:, q:q + NCOL]
    return np.ascontiguousarray(out)


# --------------------------------------------------------------------------
# bass kernel
# --------------------------------------------------------------------------

def _build_nc():
    import concourse.bacc as bacc
    import concourse.tile as tile
    import concourse.mybir as mybir

    f32 = mybir.dt.float32
    f16 = mybir.dt.float16
    Max = mybir.AluOpType.max
    X = mybir.AxisListType.X
    Copy = mybir.ActivationFunctionType.Copy

    no_ttr = bool(os.environ.get("KV_NO_TTR"))
    no_tilepos = bool(os.environ.get("KV_NO_TILEPOS"))
    nc = bacc.Bacc("TRN2", target_bir_lowering=False, debug=False,
                   num_devices=NCORES)

    # DRAM I/O -- one contiguous tensor per DMA piece (linear HBM reads)
    wts = nc.dram_tensor("wts", [128, 480], f16, kind="ExternalInput")
    plo0 = nc.dram_tensor("plo0", [128, 1025], f16, kind="ExternalInput")
    plo1 = nc.dram_tensor("plo1", [128, 1025], f16, kind="ExternalInput")
    plo2 = nc.dram_tensor("plo2", [128, 684], f16, kind="ExternalInput")
    phi01 = nc.dram_tensor("phi01", [64, 2048], f16, kind="ExternalInput")
    phi2 = nc.dram_tensor("phi2", [64, 683], f16, kind="ExternalInput")
    wavA = nc.dram_tensor("wavA", [36, NCOL], f16, kind="ExternalInput")
    wavB = nc.dram_tensor("wavB", [36, NCOL], f16, kind="ExternalInput")
    out = nc.dram_tensor("out", [128, 8], f16, kind="ExternalOutput")

    NINIT = -60000.0
    N_WARM = 3

    with tile.TileContext(nc) as tc:
        with (
            tc.tile_pool(name="sb", bufs=1) as sb,
            tc.tile_pool(name="ps", bufs=4, space="PSUM") as psp,
        ):
            # ---- SBUF tiles
            scr = sb.tile([128, 512], f16, tag="scr")
            wtsT = sb.tile([128, 480], f16, tag="wtsT")
            ploT0 = sb.tile([128, 1025], f16, tag="ploT0")
            ploT1 = sb.tile([128, 1025], f16, tag="ploT1")
            ploT2 = sb.tile([128, 684], f16, tag="ploT2")
            phiT01 = sb.tile([64, 2048], f16, tag="phiT01")
            phiT2 = sb.tile([64, 683], f16, tag="phiT2")
            wavTA = sb.tile([36, NCOL], f16, tag="wavTA")
            wavTB = sb.tile([128, NCOL], f16, tag="wavTB")  # rows 64..99 used
            cE0 = sb.tile([120, 1024], f16, tag="cE0")
            cE1 = sb.tile([120, 1024], f16, tag="cE1")
            cE2 = sb.tile([120, 683], f16, tag="cE2")
            cA0 = sb.tile([120, 1024], f16, tag="cA0")
            cA1 = sb.tile([120, 1024], f16, tag="cA1")
            fE = sb.tile([120, 1024], f16, tag="fE")
            fA = sb.tile([120, 1024], f16, tag="fA")
            out32 = sb.tile([128, 8], f16, tag="out32")

            # ---- warmup scratch init first so PE can start immediately
            nc.gpsimd.memset(scr[:], 0.0)
            nc.gpsimd.memset(out32[:], NINIT)

            # ---- input DMAs: contiguous pieces, split across SP / ACT /
            # SWDGE rings in need order (FIFO within each ring).
            nc.sync.dma_start(wtsT[:], wts[:])
            nc.sync.dma_start(ploT0[:], plo0[:])
            nc.sync.dma_start(ploT1[:], plo1[:])
            nc.sync.dma_start(ploT2[:], plo2[:])
            nc.scalar.dma_start(phiT01[:], phi01[:])
            nc.scalar.dma_start(wavTA[:], wavA[:])
            nc.scalar.dma_start(phiT2[:], phi2[:])
            nc.gpsimd.dma_start(wavTB[64:100, :], wavB[:])

            # ---- PE warmup on scratch (keeps HAM busy while DMAs land)
            psD = psp.tile([120, 1024], f32, tag="u", name="psD")
            for _ in range(N_WARM):
                nc.tensor.matmul(psD[0:80, 0:512], scr[:, 0:80], scr[:],
                                 start=True, stop=True)

            wA = wtsT[:, 0:120]          # eeg pass A (K=128)
            wC = wtsT[:, 120:240]        # eeg pass C (K=128)
            wB = wtsT[0:64, 240:360]     # eeg pass B (K=64)
            wWA = wtsT[0:36, 360:480]    # wav A lhsT
            wWB = wtsT[64:100, 360:480]  # wav B lhsT

            def eeg_unit(ps, pl, lo, n, ph, phlo):
                for j in range(0, n, 512):
                    w = min(512, n - j)
                    nc.tensor.matmul(ps[:, j:j + w], wA,
                                     pl[:, lo + j:lo + j + w],
                                     start=True, stop=False)
                    nc.tensor.matmul(ps[:, j:j + w], wC,
                                     pl[:, lo + j + 1:lo + j + 1 + w],
                                     start=False, stop=False)
                    nc.tensor.matmul(ps[:, j:j + w], wB,
                                     ph[:, phlo + j:phlo + j + w],
                                     start=False, stop=True)

            def wav_pair(psA, psB, c0, n):
                tpA = None if no_tilepos else (0, 0)
                tpB = None if no_tilepos else (64, 0)
                for j in range(0, n, 512):
                    w = min(512, n - j)
                    nc.tensor.matmul(psA[:, j:j + w], wWA,
                                     wavTA[:, c0 + j:c0 + j + w],
                                     start=True, stop=True,
                                     tile_position=tpA)
                    nc.tensor.matmul(psB[:, j:j + w], wWB,
                                     wavTB[64:100, c0 + j:c0 + j + w],
                                     start=True, stop=True,
                                     tile_position=tpB)

            # ---- matmul schedule (PSUM pool rotates 4x [120,1024] tiles)
            psE0 = psp.tile([120, 1024], f32, tag="u", name="psE0")
            eeg_unit(psE0, ploT0, 0, 1024, phiT01, 0)
            nc.scalar.activation(cE0[:], psE0[:], Copy)

            psE1 = psp.tile([120, 1024], f32, tag="u", name="psE1")
            eeg_unit(psE1, ploT1, 0, 1024, phiT01, 1024)
            nc.scalar.activation(cE1[:], psE1[:], Copy)

            psA0 = psp.tile([120, 1024], f32, tag="u", name="psA0")
            psB0 = psp.tile([120, 1024], f32, tag="u", name="psB0")
            wav_pair(psA0, psB0, 0, 1024)
            nc.scalar.activation(cA0[:], psA0[:], Copy)
            nc.vector.tensor_tensor(fE[:], cE0[:], cE1[:], Max)
            nc.vector.tensor_reduce(out32[0:120, 2:3], psB0[:], X, Max)

            psE2 = psp.tile([120, 1024], f32, tag="u", name="psE2")
            eeg_unit(psE2, ploT2, 0, 683, phiT2, 0)
            nc.scalar.activation(cE2[:], psE2[:, 0:683], Copy)

            psA1 = psp.tile([120, 1024], f32, tag="u", name="psA1")
            psB1 = psp.tile([120, 1024], f32, tag="u", name="psB1")
            wav_pair(psA1, psB1, 1024, 1024)
            nc.scalar.activation(cA1[:], psA1[:], Copy)
            nc.vector.tensor_tensor(fE[:, 0:683], fE[:, 0:683], cE2[:], Max)
            nc.vector.tensor_reduce(out32[0:120, 0:1], fE[:], X, Max)
            nc.vector.tensor_reduce(out32[0:120, 4:5], psB1[:], X, Max)

            psA2 = psp.tile([120, 1024], f32, tag="u", name="psA2")
            psB2 = psp.tile([120, 1024], f32, tag="u", name="psB2")
            wav_pair(psA2, psB2, 2048, 683)
            nc.vector.tensor_tensor(fA[:], cA0[:], cA1[:], Max)
            nc.vector.tensor_reduce(out32[0:120, 1:2], fA[:], X, Max)
            nc.vector.tensor_reduce(out32[0:120, 5:6], psA2[:, 0:683], X, Max)
            nc.vector.tensor_reduce(out32[0:120, 6:7], psB2[:, 0:683], X, Max)

            nc.sync.dma_start(out[:], out32[:])

    nc.compile()
    return nc


def _get_nc():
    if "nc" not in _NC_CACHE:
        _NC_CACHE["nc"] = _build_nc()
    return _NC_CACHE["nc"]


# --------------------------------------------------------------------------
# entry point
# --------------------------------------------------------------------------

def _prepare_in_maps(x, mu, projA_w, projB_w, conv_w):
    x = np.asarray(x, np.float32)
    eegp = np.concatenate([x[0, 0, 1:17, :], np.zeros((16, 64), np.float32)],
                          axis=1).astype(np.float16)
    zt = np.zeros(64, np.float32)
    w_padA = np.concatenate([np.zeros(7, np.float32), x[0, 0, 0, :], zt]
                            ).astype(np.float16)
    w_padB = np.concatenate([np.zeros(7, np.float32), x[0, 0, 17, :], zt]
                            ).astype(np.float16)

    conv_w = np.asarray(conv_w)
    E_A = _composite_wav_weights(mu, projA_w, conv_w[0])
    E_B = _composite_wav_weights(mu, projB_w, conv_w[2])
    WA, WC, WB = _eeg_lhsT12(conv_w[1])
    wts_np = np.zeros((128, 480), np.float16)
    wts_np[:, 0:120] = WA
    wts_np[:, 120:240] = WC
    wts_np[0:64, 240:360] = WB
    wts_np[0:36, 360:480] = _wav_lhsT(E_A)
    wts_np[64:100, 360:480] = _wav_lhsT(E_B)

    in_maps = []
    for k in range(NCORES):
        plo_k, phi_k = _eeg_phases12(eegp, k)
        in_maps.append({
            "wts": wts_np,
            "plo0": np.ascontiguousarray(plo_k[:, 0:1025]),
            "plo1": np.ascontiguousarray(plo_k[:, 1024:2049]),
            "plo2": np.ascontiguousarray(plo_k[:, 2048:2732]),
            "phi01": np.ascontiguousarray(phi_k[:, 0:2048]),
            "phi2": np.ascontiguousarray(phi_k[:, 2048:2731]),
            "wavA": _wav_phases(w_padA, k),
            "wavB": _wav_phases(w_padB, k),
        })
    return in_maps


def _head(percore, conv_b, fc1_w, fc1_b, fc2_w, fc2_b):
    m = percore.max(axis=0).astype(np.float64)            # [360]
    eeg_o = m[0:120].reshape(10, 12).max(axis=1)
    wavA_o = m[120:240].reshape(10, 12).max(axis=1)
    wavB_o = m[240:360].reshape(10, 12).max(axis=1)
    conv_b = np.asarray(conv_b, np.float64)
    f = np.concatenate([np.maximum(wavA_o + conv_b[0], 0.0),
                        np.maximum(eeg_o + conv_b[1], 0.0),
                        np.maximum(wavB_o + conv_b[2], 0.0)])
    h = 1.0 / (1.0 + np.exp(-(f @ np.asarray(fc1_w, np.float64).T
                              + np.asarray(fc1_b, np.float64))))
    o = 1.0 / (1.0 + np.exp(-(h @ np.asarray(fc2_w, np.float64).T
                              + np.asarray(fc2_b, np.float64))))
    return o[None, :].astype(np.float32)


def _percore_from_out(arr):
    """Device 'out' [128,8] fp16 -> flat [360] (eeg 120, wavA 120, wavB 120).

    cols: 0=eeg; 1,3,5=wavA; 2,4,6=wavB."""
    arr = np.asarray(arr, np.float32)
    return np.concatenate([arr[0:120, 0],
                           arr[0:120, [1, 3, 5]].max(axis=1),
                           arr[0:120, [2, 4, 6]].max(axis=1)])


def kernel(x, mu, projA_w, projB_w, conv_w, conv_b, fc1_w, fc1_b, fc2_w, fc2_b):
    global LAST_RESULT
    in_maps = _prepare_in_maps(x, mu, projA_w, projB_w, conv_w)
    nc = _get_nc()

    if os.environ.get("KERNEL_USE_SIM"):
        from concourse.bass_interp import CoreSim
        percore = np.zeros((NCORES, 360), np.float32)
        for k in range(NCORES):
            sim = CoreSim(nc)
            for name, arr in in_maps[k].items():
                sim.tensor(name)[:] = arr
            sim.simulate()
            percore[k] = _percore_from_out(sim.tensor("out"))
    else:
        from concourse.bass_utils import run_bass_kernel_spmd
        trace = bool(os.environ.get("KERNEL_TRACE"))
        res = run_bass_kernel_spmd(nc, in_maps, list(range(NCORES)),
                                   trace=trace)
        LAST_RESULT = res
        percore = np.stack([_percore_from_out(res.results[k]["out"])
                            for k in range(NCORES)])

    return _head(percore, conv_b, fc1_w, fc1_b, fc2_w, fc2_b)


# --------------------------------------------------------------------------
# numpy self-check of the host-side math (no hardware needed)
# --------------------------------------------------------------------------

def _selfcheck():
    rng = np.random.default_rng(0)
    Tm = 12 * (NCOL + 2) + 64
    eeg = rng.standard_normal((16, T)).astype(np.float32)
    W1 = (rng.standard_normal((10, 16, 9)) * 0.1).astype(np.float32)

    # reference conv for a window
    k = 3
    s = _core_start(k)
    ref = np.zeros((10, TC))
    for j in range(9):
        ref += np.einsum('oc,ct->ot', W1[:, :, j],
                         eeg[:, s + j:s + j + TC])

    eegp = np.concatenate([eeg, np.zeros((16, 64), np.float32)], axis=1)
    plo_k, phi_k = _eeg_phases12(eegp, k)
    WA, WC, WB = _eeg_lhsT12(W1)

    # emulate the three passes
    got = np.zeros((120, NCOL))
    got += WA.T @ plo_k[:, 0:NCOL]
    got += WC.T @ plo_k[:, 1:NCOL + 1]
    got += WB.T @ phi_k[:, 0:NCOL]
    got_ot = got.reshape(10, 12, NCOL).transpose(0, 2, 1).reshape(10, TC)
    err = np.abs(got_ot - ref).max()
    print("eeg B=12 max err:", err)
    assert err < 2e-2, err

    # wav path check
    wav = rng.standard_normal(T).astype(np.float32)
    E = rng.standard_normal((10, 23)) * 0.1
    w_pad = np.concatenate([np.zeros(7, np.float32), wav,
                            np.zeros(64, np.float32)]).astype(np.float16)
    ph = _wav_phases(w_pad, k)
    L = _wav_lhsT(E)
    gotw = (L.T @ ph.astype(np.float64)).reshape(10, 12, NCOL)
    gotw = gotw.transpose(0, 2, 1).reshape(10, TC)
    refw = np.zeros((10, TC))
    wp = np.concatenate([np.zeros(7), wav.astype(np.float64)])
    for j in range(23):
        refw += np.outer(E[:, j], wp[s + j:s + j + TC])
    errw = np.abs(gotw - refw).max()
    print("wav max err:", errw)
    assert errw < 2e-2, errw
    print("selfcheck OK")


if __name__ == "__main__":
    _selfcheck()


# revision 14
# speedup vs baseline: 1.0614x; 1.0482x over previous
"""Trainium2 Bass kernel for nn_CNN_88098369175781.

Model: x[1,1,18,T=262144] -> wavA=x[...,0,:], eeg=x[...,1:17,:], wavB=x[...,17,:]
  wav streams: proj(1->16, pointwise) -> diagonal sinc filter bank (15 taps,
  pad 7) -> conv(16->10, 9 taps) + bias -> relu -> global max-pool.
  eeg stream:  conv(16->10, 9 taps) + bias -> relu -> global max-pool.
  concat -> sigmoid FC(30->30) -> sigmoid FC(30->2).

Device decomposition:
  * Wav streams fuse to ONE 1->10ch 23-tap conv (precomposed host-side).
  * Bias/relu commute past the global max; device computes convs + maxima.
  * eeg conv via B=12 polyphase: out[o, s+12m+dt], M=120=(o,dt), three
    accumulating passes: A (s'=r in [0,8), K=128), C (s'=12+r from the
    m+1 column, K=128), B (s'=8+r' in [8,12), K=64).
  * wav via 12-phase polyphase (K=36, M=120); the A and B streams run as
    row-tiled concurrent matmuls (tile_position (0,0) / (64,0)).
  * PSUM evacuation: ACT casts some banks to fp16 SBUF; GpSimd folds the
    stream tails into those casts in-place (tensor_tensor max); DVE
    tensor_tensor_reduce consumes a fresh PSUM bank and a cast tile per
    pass, writing per-row maxima into the output tile.
  * 8 cores split the time axis (overlapping chunks; overlap free for max).
  * Host combines per-core maxima and runs the tiny FC head.
"""

import os
import numpy as np

T = 262144
NOUT = T - 8            # 262136 valid conv output positions
NCORES = 8
KLEN = 15
SIGMA = 0.005

B12 = 12
NCOL = 2731             # phase columns per core (12*2731 = 32772 outputs)
TC = 12 * NCOL          # outputs per core

_NC_CACHE = {}
LAST_RESULT = None      # BassKernelResults of the most recent device run


# --------------------------------------------------------------------------
# host-side weight precompute
# --------------------------------------------------------------------------

def _sinc_rows(mu):
    """Diagonal rows of the reference's sinc_kernel: [16, 15] float64."""
    k = np.linspace(-1.0, 1.0, KLEN)
    kk = (k[None, :] - np.asarray(mu, np.float64)[:, None]) / SIGMA
    nos = np.sum(np.abs(kk) < 1e-5, axis=1)
    kk = np.where((nos >= 0.5)[:, None], kk - 5e-5, kk)
    return np.sin(np.pi * kk) / (np.pi * kk)


def _composite_wav_weights(mu, proj_w, conv_w_i):
    """Fused 1->10ch 23-tap kernel E[o, s] (float64)."""
    krn = _sinc_rows(mu)                                  # [16,15]
    a = np.asarray(proj_w, np.float64)[:, 0, 0]           # [16]
    W = np.asarray(conv_w_i, np.float64)                  # [10,16,9]
    E = np.zeros((10, 23))
    for j in range(9):
        E[:, j:j + 15] += np.einsum('oc,cm->om', W[:, :, j] * a[None, :], krn)
    return E


def _eeg_lhsT12(W1):
    """B=12 eeg weights: (W_A [128,120], W_C [128,120], W_B [64,120]).

    cols (o*12+dt); A rows (c*8+r): W1[o,c,r-dt]; C rows (c*8+r):
    W1[o,c,12+r-dt]; B rows (c*4+r'): W1[o,c,8+r'-dt]."""
    W1 = np.asarray(W1, np.float64)         # [10,16,9]
    WA = np.zeros((128, 120))
    WC = np.zeros((128, 120))
    WB = np.zeros((64, 120))
    for o in range(10):
        for dt in range(12):
            col = o * 12 + dt
            for c in range(16):
                for r in range(8):
                    j = r - dt
                    if 0 <= j < 9:
                        WA[c * 8 + r, col] = W1[o, c, j]
                    j = 12 + r - dt
                    if 0 <= j < 9:
                        WC[c * 8 + r, col] = W1[o, c, j]
                for rp in range(4):
                    j = 8 + rp - dt
                    if 0 <= j < 9:
                        WB[c * 4 + rp, col] = W1[o, c, j]
    return (WA.astype(np.float32), WC.astype(np.float32),
            WB.astype(np.float32))


def _wav_lhsT(E):
    """[36, 120]: row v*3+q, col o*12+dt, val E[o, 12q+v-dt]."""
    out = np.zeros((36, 120))
    v, q, o, dt = np.meshgrid(np.arange(12), np.arange(3), np.arange(10),
                              np.arange(12), indexing='ij')
    s = 12 * q + v - dt
    valid = (s >= 0) & (s < 23)
    out[(v * 3 + q)[valid], (o * 12 + dt)[valid]] = E[o[valid], np.clip(s[valid], 0, 22)]
    return out.astype(np.float32)


# --------------------------------------------------------------------------
# host-side per-core input slicing
# --------------------------------------------------------------------------

def _core_start(k):
    return min(k * 32767, NOUT - TC)


def _eeg_phases12(eegp, k):
    """eegp: [16, T+pad] fp16. Returns (plo [128, 2732], phi [64, 2731])."""
    s = _core_start(k)
    v = eegp[:, s:s + 12 * (NCOL + 1)]                    # [16, 32784]
    p = v.reshape(16, NCOL + 1, 12)                       # [16, 2732, 12]
    plo = p[:, :, 0:8].transpose(0, 2, 1).reshape(128, NCOL + 1)
    phi = p[:, 0:NCOL, 8:12].transpose(0, 2, 1).reshape(64, NCOL)
    return np.ascontiguousarray(plo), np.ascontiguousarray(phi)


def _wav_phases(w_pad, k):
    """[36, 2731]: row v*3+q, col n = w_pad[s + 12(n+q) + v]."""
    s = _core_start(k)
    sl = w_pad[s:s + 12 * (NCOL + 2)]
    y = sl.reshape(NCOL + 2, 12).T                        # y[v,m] = sl[12m+v]
    out = np.empty((36, NCOL), dtype=w_pad.dtype)
    for q in range(3):
        out[q::3, :] = y[:, q:q + NCOL]
    return np.ascontiguousarray(out)


# --------------------------------------------------------------------------
# bass kernel
# --------------------------------------------------------------------------

def _build_nc():
    import concourse.bacc as bacc
    import concourse.tile as tile
    import concourse.mybir as mybir

    f32 = mybir.dt.float32
    f16 = mybir.dt.float16
    Max = mybir.AluOpType.max
    X = mybir.AxisListType.X
    Copy = mybir.ActivationFunctionType.Copy

    no_ttr = bool(os.environ.get("KV_NO_TTR"))
    no_tilepos = bool(os.environ.get("KV_NO_TILEPOS"))
    nc = bacc.Bacc("TRN2", target_bir_lowering=False, debug=False,
                   num_devices=NCORES)

    # DRAM I/O -- one contiguous tensor per DMA piece (linear HBM reads)
    wts = nc.dram_tensor("wts", [128, 480], f16, kind="ExternalInput")
    plo0 = nc.dram_tensor("plo0", [128, 1025], f16, kind="ExternalInput")
    plo1 = nc.dram_tensor("plo1", [128, 1025], f16, kind="ExternalInput")
    plo2 = nc.dram_tensor("plo2", [128, 684], f16, kind="ExternalInput")
    phi01 = nc.dram_tensor("phi01", [64, 2048], f16, kind="ExternalInput")
    phi2 = nc.dram_tensor("phi2", [64, 683], f16, kind="ExternalInput")
    wavA = nc.dram_tensor("wavA", [36, NCOL], f16, kind="ExternalInput")
    wavB = nc.dram_tensor("wavB", [36, NCOL], f16, kind="ExternalInput")
    out = nc.dram_tensor("out", [128, 8], f16, kind="ExternalOutput")

    NINIT = -60000.0
    N_WARM = 3

    with tile.TileContext(nc) as tc:
        with (
            tc.tile_pool(name="sb", bufs=1) as sb,
            tc.tile_pool(name="ps", bufs=4, space="PSUM") as psp,
        ):
            # ---- SBUF tiles
            scr = sb.tile([128, 512], f16, tag="scr")
            wtsT = sb.tile([128, 480], f16, tag="wtsT")
            ploT0 = sb.tile([128, 1025], f16, tag="ploT0")
            ploT1 = sb.tile([128, 1025], f16, tag="ploT1")
            ploT2 = sb.tile([128, 684], f16, tag="ploT2")
            phiT01 = sb.tile([64, 2048], f16, tag="phiT01")
            phiT2 = sb.tile([64, 683], f16, tag="phiT2")
            wavTA = sb.tile([36, NCOL], f16, tag="wavTA")
            wavTB = sb.tile([128, NCOL], f16, tag="wavTB")  # rows 64..99 used
            cE0 = sb.tile([120, 1024], f16, tag="cE0")
            cE1 = sb.tile([120, 1024], f16, tag="cE1")
            cE2 = sb.tile([120, 683], f16, tag="cE2")
            cA0 = sb.tile([120, 1024], f16, tag="cA0")
            cA1 = sb.tile([120, 1024], f16, tag="cA1")
            fE = sb.tile([120, 1024], f16, tag="fE")
            fA = sb.tile([120, 1024], f16, tag="fA")
            out32 = sb.tile([128, 8], f16, tag="out32")

            # ---- warmup scratch init first so PE can start immediately
            nc.gpsimd.memset(scr[:], 0.0)
            nc.gpsimd.memset(out32[:], NINIT)

            # ---- input DMAs: contiguous pieces, split across SP / ACT /
            # SWDGE rings in need order (FIFO within each ring).
            nc.sync.dma_start(ploT0[:], plo0[:])
            nc.sync.dma_start(ploT1[:], plo1[:])
            nc.sync.dma_start(ploT2[:], plo2[:])
            nc.scalar.dma_start(wtsT[:], wts[:])
            nc.scalar.dma_start(phiT01[:], phi01[:])
            nc.scalar.dma_start(phiT2[:], phi2[:])
            nc.gpsimd.dma_start(wavTA[:], wavA[:])
            nc.gpsimd.dma_start(wavTB[64:100, :], wavB[:])

            # ---- PE warmup on scratch (keeps HAM busy while DMAs land)
            psD = psp.tile([120, 1024], f32, tag="u", name="psD")
            for _ in range(N_WARM):
                nc.tensor.matmul(psD[0:80, 0:512], scr[:, 0:80], scr[:],
                                 start=True, stop=True)

            wA = wtsT[:, 0:120]          # eeg pass A (K=128)
            wC = wtsT[:, 120:240]        # eeg pass C (K=128)
            wB = wtsT[0:64, 240:360]     # eeg pass B (K=64)
            wWA = wtsT[0:36, 360:480]    # wav A lhsT
            wWB = wtsT[64:100, 360:480]  # wav B lhsT

            def eeg_unit(ps, pl, lo, n, ph, phlo):
                for j in range(0, n, 512):
                    w = min(512, n - j)
                    nc.tensor.matmul(ps[:, j:j + w], wA,
                                     pl[:, lo + j:lo + j + w],
                                     start=True, stop=False)
                    nc.tensor.matmul(ps[:, j:j + w], wC,
                                     pl[:, lo + j + 1:lo + j + 1 + w],
                                     start=False, stop=False)
                    nc.tensor.matmul(ps[:, j:j + w], wB,
                                     ph[:, phlo + j:phlo + j + w],
                                     start=False, stop=True)

            def wav_pair(psA, psB, c0, n):
                tpA = None if no_tilepos else (0, 0)
                tpB = None if no_tilepos else (64, 0)
                for j in range(0, n, 512):
                    w = min(512, n - j)
                    nc.tensor.matmul(psA[:, j:j + w], wWA,
                                     wavTA[:, c0 + j:c0 + j + w],
                                     start=True, stop=True,
                                     tile_position=tpA)
                    nc.tensor.matmul(psB[:, j:j + w], wWB,
                                     wavTB[64:100, c0 + j:c0 + j + w],
                                     start=True, stop=True,
                                     tile_position=tpB)

            # ---- matmul schedule (PSUM pool rotates 4x [120,1024] tiles)
            psE0 = psp.tile([120, 1024], f32, tag="u", name="psE0")
            eeg_unit(psE0, ploT0, 0, 1024, phiT01, 0)
            nc.scalar.activation(cE0[:], psE0[:], Copy)

            psE1 = psp.tile([120, 1024], f32, tag="u", name="psE1")
            eeg_unit(psE1, ploT1, 0, 1024, phiT01, 1024)
            nc.scalar.activation(cE1[:], psE1[:], Copy)

            psA0 = psp.tile([120, 1024], f32, tag="u", name="psA0")
            psB0 = psp.tile([120, 1024], f32, tag="u", name="psB0")
            wav_pair(psA0, psB0, 0, 1024)
            nc.scalar.activation(cA0[:], psA0[:], Copy)
            nc.vector.tensor_tensor(fE[:], cE0[:], cE1[:], Max)
            nc.vector.tensor_reduce(out32[0:120, 2:3], psB0[:], X, Max)

            psE2 = psp.tile([120, 1024], f32, tag="u", name="psE2")
            eeg_unit(psE2, ploT2, 0, 683, phiT2, 0)
            nc.scalar.activation(cE2[:], psE2[:, 0:683], Copy)

            psA1 = psp.tile([120, 1024], f32, tag="u", name="psA1")
            psB1 = psp.tile([120, 1024], f32, tag="u", name="psB1")
            wav_pair(psA1, psB1, 1024, 1024)
            nc.scalar.activation(cA1[:], psA1[:], Copy)
            nc.vector.tensor_tensor(fE[:, 0:683], fE[:, 0:683], cE2[:], Max)
            nc.vector.tensor_reduce(out32[0:120, 0:1], fE[:], X, Max)
            nc.vector.tensor_reduce(out32[0:120, 4:5], psB1[:], X, Max)

            psA2 = psp.tile([120, 1024], f32, tag="u", name="psA2")
            psB2 = psp.tile([120, 1024], f32, tag="u", name="psB2")
            wav_pair(psA2, psB2, 2048, 683)
            nc.vector.tensor_tensor(fA[:], cA0[:], cA1[:], Max)
            nc.vector.tensor_reduce(out32[0:120, 1:2], fA[:], X, Max)
            nc.vector.tensor_reduce(out32[0:120, 5:6], psA2[:, 0:683], X, Max)
            nc.vector.tensor_reduce(out32[0:120, 6:7], psB2[:, 0:683], X, Max)

            nc.sync.dma_start(out[:], out32[:])

    nc.compile()
    return nc


def _get_nc():
    if "nc" not in _NC_CACHE:
        _NC_CACHE["nc"] = _build_nc()
    return _NC_CACHE["nc"]


# --------------------------------------------------------------------------
# entry point
# --------------------------------------------------------------------------

def _prepare_in_maps(x, mu, projA_w, projB_w, conv_w):
    x = np.asarray(x, np.float32)
    eegp = np.concatenate([x[0, 0, 1:17, :], np.zeros((16, 64), np.float32)],
                          axis=1).astype(np.float16)
    zt = np.zeros(64, np.float32)
    w_padA = np.concatenate([np.zeros(7, np.float32), x[0, 0, 0, :], zt]
                            ).astype(np.float16)
    w_padB = np.concatenate([np.zeros(7, np.float32), x[0, 0, 17, :], zt]
                            ).astype(np.float16)

    conv_w = np.asarray(conv_w)
    E_A = _composite_wav_weights(mu, projA_w, conv_w[0])
    E_B = _composite_wav_weights(mu, projB_w, conv_w[2])
    WA, WC, WB = _eeg_lhsT12(conv_w[1])
    wts_np = np.zeros((128, 480), np.float16)
    wts_np[:, 0:120] = WA
    wts_np[:, 120:240] = WC
    wts_np[0:64, 240:360] = WB
    wts_np[0:36, 360:480] = _wav_lhsT(E_A)
    wts_np[64:100, 360:480] = _wav_lhsT(E_B)

    in_maps = []
    for k in range(NCORES):
        plo_k, phi_k = _eeg_phases12(eegp, k)
        in_maps.append({
            "wts": wts_np,
            "plo0": np.ascontiguousarray(plo_k[:, 0:1025]),
            "plo1": np.ascontiguousarray(plo_k[:, 1024:2049]),
            "plo2": np.ascontiguousarray(plo_k[:, 2048:2732]),
            "phi01": np.ascontiguousarray(phi_k[:, 0:2048]),
            "phi2": np.ascontiguousarray(phi_k[:, 2048:2731]),
            "wavA": _wav_phases(w_padA, k),
            "wavB": _wav_phases(w_padB, k),
        })
    return in_maps


def _head(percore, conv_b, fc1_w, fc1_b, fc2_w, fc2_b):
    m = percore.max(axis=0).astype(np.float64)            # [360]
    eeg_o = m[0:120].reshape(10, 12).max(axis=1)
    wavA_o = m[120:240].reshape(10, 12).max(axis=1)
    wavB_o = m[240:360].reshape(10, 12).max(axis=1)
    conv_b = np.asarray(conv_b, np.float64)
    f = np.concatenate([np.maximum(wavA_o + conv_b[0], 0.0),
                        np.maximum(eeg_o + conv_b[1], 0.0),
                        np.maximum(wavB_o + conv_b[2], 0.0)])
    h = 1.0 / (1.0 + np.exp(-(f @ np.asarray(fc1_w, np.float64).T
                              + np.asarray(fc1_b, np.float64))))
    o = 1.0 / (1.0 + np.exp(-(h @ np.asarray(fc2_w, np.float64).T
                              + np.asarray(fc2_b, np.float64))))
    return o[None, :].astype(np.float32)


def _percore_from_out(arr):
    """Device 'out' [128,8] fp16 -> flat [360] (eeg 120, wavA 120, wavB 120).

    cols: 0=eeg; 1,3,5=wavA; 2,4,6=wavB."""
    arr = np.asarray(arr, np.float32)
    return np.concatenate([arr[0:120, 0],
                           arr[0:120, [1, 3, 5]].max(axis=1),
                           arr[0:120, [2, 4, 6]].max(axis=1)])


def kernel(x, mu, projA_w, projB_w, conv_w, conv_b, fc1_w, fc1_b, fc2_w, fc2_b):
    global LAST_RESULT
    in_maps = _prepare_in_maps(x, mu, projA_w, projB_w, conv_w)
    nc = _get_nc()

    if os.environ.get("KERNEL_USE_SIM"):
        from concourse.bass_interp import CoreSim
        percore = np.zeros((NCORES, 360), np.float32)
        for k in range(NCORES):
            sim = CoreSim(nc)
            for name, arr in in_maps[k].items():
                sim.tensor(name)[:] = arr
            sim.simulate()
            percore[k] = _percore_from_out(sim.tensor("out"))
    else:
        from concourse.bass_utils import run_bass_kernel_spmd
        trace = bool(os.environ.get("KERNEL_TRACE"))
        res = run_bass_kernel_spmd(nc, in_maps, list(range(NCORES)),
                                   trace=trace)
        LAST_RESULT = res
        percore = np.stack([_percore_from_out(res.results[k]["out"])
                            for k in range(NCORES)])

    return _head(percore, conv_b, fc1_w, fc1_b, fc2_w, fc2_b)


# --------------------------------------------------------------------------
# numpy self-check of the host-side math (no hardware needed)
# --------------------------------------------------------------------------

def _selfcheck():
    rng = np.random.default_rng(0)
    Tm = 12 * (NCOL + 2) + 64
    eeg = rng.standard_normal((16, T)).astype(np.float32)
    W1 = (rng.standard_normal((10, 16, 9)) * 0.1).astype(np.float32)

    # reference conv for a window
    k = 3
    s = _core_start(k)
    ref = np.zeros((10, TC))
    for j in range(9):
        ref += np.einsum('oc,ct->ot', W1[:, :, j],
                         eeg[:, s + j:s + j + TC])

    eegp = np.concatenate([eeg, np.zeros((16, 64), np.float32)], axis=1)
    plo_k, phi_k = _eeg_phases12(eegp, k)
    WA, WC, WB = _eeg_lhsT12(W1)

    # emulate the three passes
    got = np.zeros((120, NCOL))
    got += WA.T @ plo_k[:, 0:NCOL]
    got += WC.T @ plo_k[:, 1:NCOL + 1]
    got += WB.T @ phi_k[:, 0:NCOL]
    got_ot = got.reshape(10, 12, NCOL).transpose(0, 2, 1).reshape(10, TC)
    err = np.abs(got_ot - ref).max()
    print("eeg B=12 max err:", err)
    assert err < 2e-2, err

    # wav path check
    wav = rng.standard_normal(T).astype(np.float32)
    E = rng.standard_normal((10, 23)) * 0.1
    w_pad = np.concatenate([np.zeros(7, np.float32), wav,
                            np.zeros(64, np.float32)]).astype(np.float16)
    ph = _wav_phases(w_pad, k)
    L = _wav_lhsT(E)
    gotw = (L.T @ ph.astype(np.float64)).reshape(10, 12, NCOL)
    gotw = gotw.transpose(0, 2, 1).reshape(10, TC)
    refw = np.zeros((10, TC))
    wp = np.concatenate([np.zeros(7), wav.astype(np.float64)])
    for j in range(23):
        refw += np.outer(E[:, j], wp[s + j:s + j + TC])
    errw = np.abs(gotw - refw).max()
    print("wav max err:", errw)
    assert errw < 2e-2, errw
    print("selfcheck OK")


if __name__ == "__main__":
    _selfcheck()


# revision 15
# speedup vs baseline: 1.1298x; 1.0645x over previous
"""Trainium2 Bass kernel for nn_CNN_88098369175781.

Model: x[1,1,18,T=262144] -> wavA=x[...,0,:], eeg=x[...,1:17,:], wavB=x[...,17,:]
  wav streams: proj(1->16, pointwise) -> diagonal sinc filter bank (15 taps,
  pad 7) -> conv(16->10, 9 taps) + bias -> relu -> global max-pool.
  eeg stream:  conv(16->10, 9 taps) + bias -> relu -> global max-pool.
  concat -> sigmoid FC(30->30) -> sigmoid FC(30->2).

Device decomposition (validated vs reference in numpy):
  * Each wav stream's three linear stages fuse into ONE 1->10 channel, 23-tap
    conv on the zero-padded raw wav signal (weights precomposed on host).
  * Bias/relu commute past the global max (bias is constant over time;
    max(relu(h)) = relu(max(h))), so the device only computes convs + maxima.
  * Convs run on the tensor engine via a polyphase formulation:
      eeg:  time phases r in [0,8), outputs (o, dt in [0,8)) => M=80,
            contraction (c,r) => K=128, 2 accumulating matmuls (u-groups).
      wav:  time phases v in [0,12), outputs (o, dt in [0,12)) => M=120,
            contraction (v,q in [0,3)) => K=36, single matmul per tile
            (the q-replication is materialized host-side).  The A and B
            streams run as row-tiled CONCURRENT matmuls: A-phases live at
            partitions 0:36 (tile_position (0,0)), B-phases at partitions
            64:100 (tile_position (64,0)), halving wav PE time.
  * Operands are fp16 (PSUM accumulation stays fp32): fp32 matmuls run as two
    HW passes on trn2, fp16 single-pass -- and DMA bytes halve.
  * Max-reduction is split across engines: the vector engine reduces eeg PSUM
    directly (fp32); the scalar engine casts wav PSUM to fp16 in SBUF and the
    vector engine reduces those.
  * 8 cores split the time axis (overlapping chunks; overlap is free for max).
  * Host combines per-core maxima and runs the tiny FC head.
"""

import os
import numpy as np

T = 262144
NOUT = T - 8            # 262136 valid conv output positions
NCORES = 8
KLEN = 15
SIGMA = 0.005

EEG_NCOL = 4096         # eeg matmul columns per core (8 outputs each)
EEG_COLS = EEG_NCOL + 1  # phase row length (g=1 needs one extra column)
WAV_NCOL = 2731         # wav matmul columns per core (12 outputs each)
EEG_TC = 8 * EEG_NCOL   # 32768 eeg outputs per core
WAV_TC = 12 * WAV_NCOL  # 32772 wav outputs per core

_NC_CACHE = {}
LAST_RESULT = None      # BassKernelResults of the most recent device run


# --------------------------------------------------------------------------
# host-side weight precompute
# --------------------------------------------------------------------------

def _sinc_rows(mu):
    """Diagonal rows of the reference's sinc_kernel: [16, 15] float64."""
    k = np.linspace(-1.0, 1.0, KLEN)
    kk = (k[None, :] - np.asarray(mu, np.float64)[:, None]) / SIGMA
    nos = np.sum(np.abs(kk) < 1e-5, axis=1)
    kk = np.where((nos >= 0.5)[:, None], kk - 5e-5, kk)
    return np.sin(np.pi * kk) / (np.pi * kk)


def _composite_wav_weights(mu, proj_w, conv_w_i):
    """Fused 1->10ch 23-tap kernel E[o, s] (float64)."""
    krn = _sinc_rows(mu)                                  # [16,15]
    a = np.asarray(proj_w, np.float64)[:, 0, 0]           # [16]
    W = np.asarray(conv_w_i, np.float64)                  # [10,16,9]
    E = np.zeros((10, 23))
    for j in range(9):
        E[:, j:j + 15] += np.einsum('oc,cm->om', W[:, :, j] * a[None, :], krn)
    return E


def _eeg_lhsT(W1):
    """[128, 160]: cols g*80+(o*8+dt); row c*8+r; val W1[o,c,8g+r-dt]."""
    W1 = np.asarray(W1, np.float64)
    out = np.zeros((128, 160))
    g, c, r, o, dt = np.meshgrid(np.arange(2), np.arange(16), np.arange(8),
                                 np.arange(10), np.arange(8), indexing='ij')
    j = 8 * g + r - dt
    valid = (j >= 0) & (j < 9)
    out[(c * 8 + r)[valid], (g * 80 + o * 8 + dt)[valid]] = \
        W1[o[valid], c[valid], np.clip(j[valid], 0, 8)]
    return out.astype(np.float32)


def _wav_lhsT(E):
    """[36, 120]: row v*3+q, col o*12+dt, val E[o, 12q+v-dt]."""
    out = np.zeros((36, 120))
    v, q, o, dt = np.meshgrid(np.arange(12), np.arange(3), np.arange(10),
                              np.arange(12), indexing='ij')
    s = 12 * q + v - dt
    valid = (s >= 0) & (s < 23)
    out[(v * 3 + q)[valid], (o * 12 + dt)[valid]] = E[o[valid], np.clip(s[valid], 0, 22)]
    return out.astype(np.float32)


# --------------------------------------------------------------------------
# host-side per-core input slicing
# --------------------------------------------------------------------------

def _core_starts(k):
    return (min(k * 32767, NOUT - EEG_TC), min(k * 32767, NOUT - WAV_TC))


def _eeg_phases(eeg, k):
    """[128, 4097]: row c*8+r, col m = eeg[c, s_e + 8m + r]."""
    s_e, _ = _core_starts(k)
    v = eeg[:, s_e:s_e + 8 * EEG_COLS]                  # [16, 32776]
    p = v.reshape(16, EEG_COLS, 8).transpose(0, 2, 1)   # [16,8,4097]
    return p.reshape(128, EEG_COLS)


def _wav_phases(w_pad, k):
    """[36, 2731]: row v*3+q, col n = w_pad[s_w + 12(n+q) + v]."""
    _, s_w = _core_starts(k)
    sl = w_pad[s_w:s_w + 12 * (WAV_NCOL + 2)]
    y = sl.reshape(WAV_NCOL + 2, 12).T                  # y[v,m] = sl[12m+v]
    out = np.empty((36, WAV_NCOL), dtype=w_pad.dtype)
    for q in range(3):
        out[q::3, :] = y[:, q:q + WAV_NCOL]
    return np.ascontiguousarray(out)


# --------------------------------------------------------------------------
# bass kernel
# --------------------------------------------------------------------------

def _build_nc():
    import concourse.bacc as bacc
    import concourse.tile as tile
    import concourse.mybir as mybir

    f32 = mybir.dt.float32
    f16 = mybir.dt.float16
    nc = bacc.Bacc("TRN2", target_bir_lowering=False, debug=False,
                   num_devices=NCORES)

    eegP = nc.dram_tensor("eegP", [128, EEG_COLS], f16, kind="ExternalInput")
    wavPA = nc.dram_tensor("wavPA", [36, WAV_NCOL], f16, kind="ExternalInput")
    wavPB = nc.dram_tensor("wavPB", [36, WAV_NCOL], f16, kind="ExternalInput")
    wts = nc.dram_tensor("wts", [128, 280], f16, kind="ExternalInput")
    out = nc.dram_tensor("out", [128, 10], f16, kind="ExternalOutput")

    N_ECHUNK = 2                 # eeg input loaded in 2 column chunks
    ECHUNK = 2048                # chunk j covers cols [2048j, 2048j+2049)
    N_WARM = 6                   # dummy matmuls to warm the PE clock gate

    with tile.TileContext(nc) as tc:
        with (
            tc.tile_pool(name="sb", bufs=1) as sb,
            tc.tile_pool(name="ps", bufs=4, space="PSUM") as psp,
        ):
            # PE warmup: dummy matmuls on a zeroed scratch tile keep the PE
            # busy while the first input DMAs land, so the HAM clock-gate
            # opens (1.2 -> 2.4 GHz) before the real matmuls start.
            scr = sb.tile([128, 512], f16, tag="scr")
            nc.gpsimd.memset(scr[:], 0.0)
            wps = psp.tile([120, 1024], f32, tag="ps", name="wps")
            for _ in range(N_WARM):
                nc.tensor.matmul(wps[0:80, 0:512], scr[:, 0:80], scr[:],
                                 start=True, stop=True)

            # input DMAs spread over three issue engines: descriptor
            # generation costs ~0.7us per dma_start and serializes per engine
            echunks = [sb.tile([128, ECHUNK + 1], f16, tag=f"eegchunk{j}",
                               name=f"eegchunk{j}") for j in range(N_ECHUNK)]
            wts_t = sb.tile([128, 280], f16, tag="wts")
            # wav phases: A rows 0:36, B rows 64:100 (for row-tiled pairs)
            wav_t = sb.tile([128, WAV_NCOL], f16, tag="wav")
            nc.scalar.dma_start(wts_t[:], wts[:])
            nc.sync.dma_start(echunks[0][:], eegP[:, 0:ECHUNK + 1])
            nc.sync.dma_start(echunks[1][:], eegP[:, ECHUNK:2 * ECHUNK + 1])
            nc.scalar.dma_start(wav_t[0:36, :], wavPA[:])
            nc.scalar.dma_start(wav_t[64:100, :], wavPB[:])
            wE_t = wts_t[:, 0:160]
            wWA = wts_t[0:36, 160:280]
            wWB = wts_t[64:100, 160:280]

            # merged fp16 output tile; host finishes the max over columns
            # cols: 0=eeg p0, 1=eeg p2, 2=wavA tail, 3=wavB tail (fp32 path),
            #       4=eeg p1, 5=eeg p3, 6,7=wavB p0,p1, 8,9=wavA p0,p1 (fp16)
            out16 = sb.tile([128, 10], f16, tag="out16")
            nc.gpsimd.memset(out16[:], 0.0)
            # fp32 maxima landing tile for the PSUM-direct reduces
            mF = sb.tile([120, 4], f32, tag="mF")
            nc.gpsimd.memset(mF[:], 0.0)
            # fp16 staging tiles for the ACT-evacuated psum pairs
            stg = [sb.tile([120, 1024], f16, tag=f"stg{i}", name=f"stg{i}")
                   for i in range(4)]

            X = mybir.AxisListType.X
            Copy = mybir.ActivationFunctionType.Copy

            # PSUM evacuation is split: ACT casts some banks to fp16 SBUF
            # ((N+352)/1.2 per op) while DVE direct-reduces the others from
            # PSUM (1 elem/cycle), then reduces the fp16 staged tiles.

            # eeg: pairs 0,2 -> DVE direct; pairs 1,3 -> ACT cast
            for p in range(4):
                ch = echunks[p // 2]
                base = (p % 2) * 1024
                ps = psp.tile([120, 1024], f32, tag="ps", name=f"pse{p}")
                for g in range(2):
                    for j in range(2):
                        lo = j * 512
                        nc.tensor.matmul(ps[0:80, lo:lo + 512],
                                         wE_t[:, 80 * g:80 * g + 80],
                                         ch[:, base + lo + g:base + lo + g + 512],
                                         start=(g == 0), stop=(g == 1))
                if p % 2 == 0:
                    nc.vector.reduce_max(mF[0:80, p // 2:p // 2 + 1],
                                         ps[0:80, :], axis=X)
                else:
                    nc.scalar.activation(stg[p // 2][0:80, :], ps[0:80, :], Copy)
            nc.vector.reduce_max(out16[0:80, 4:5], stg[0][0:80, :], axis=X)
            nc.vector.reduce_max(out16[0:80, 5:6], stg[1][0:80, :], axis=X)

            # wav: A/B row-tiled concurrent pairs; per column-chunk p the A
            # and B matmuls run in different PE row groups simultaneously.
            # chunks p=0,1: A -> ACT cast + fp16 reduce, B -> DVE direct;
            # tail chunk (683 cols): both DVE direct from PSUM.
            for p in range(3):
                psA = psp.tile([120, 1024], f32, tag="ps", name=f"pswA{p}")
                psB = psp.tile([120, 1024], f32, tag="ps", name=f"pswB{p}")
                for j in range(2):
                    n0 = (2 * p + j) * 512
                    nn = min(512, WAV_NCOL - n0)
                    nc.tensor.matmul(psA[:, j * 512:j * 512 + nn],
                                     wWA, wav_t[0:36, n0:n0 + nn],
                                     start=True, stop=True,
                                     tile_position=(0, 0))
                    nc.tensor.matmul(psB[:, j * 512:j * 512 + nn],
                                     wWB, wav_t[64:100, n0:n0 + nn],
                                     start=True, stop=True,
                                     tile_position=(64, 0))
                if p < 2:
                    nc.scalar.activation(stg[2][:] if p == 0 else stg[3][:],
                                         psA[:], Copy)
                    nc.vector.reduce_max(out16[0:120, 6 + p:7 + p],
                                         psB[:], axis=X)
                else:
                    nc.vector.reduce_max(mF[:, 2:3], psA[:, 0:683], axis=X)
                    nc.vector.reduce_max(mF[:, 3:4], psB[:, 0:683], axis=X)
            nc.vector.reduce_max(out16[0:120, 8:9], stg[2][:], axis=X)
            nc.vector.reduce_max(out16[0:120, 9:10], stg[3][:], axis=X)

            # cast the fp32 maxima into the merged fp16 output tile
            nc.vector.tensor_copy(out16[0:120, 0:4], mF[:])

            nc.sync.dma_start(out[:], out16[:])

    nc.compile()
    return nc


def _get_nc():
    if "nc" not in _NC_CACHE:
        _NC_CACHE["nc"] = _build_nc()
    return _NC_CACHE["nc"]


# --------------------------------------------------------------------------
# entry point
# --------------------------------------------------------------------------

def _prepare_in_maps(x, mu, projA_w, projB_w, conv_w):
    x = np.asarray(x, np.float32)
    eeg = np.ascontiguousarray(x[0, 0, 1:17, :]).astype(np.float16)
    zt = np.zeros(64, np.float32)
    w_padA = np.concatenate([np.zeros(7, np.float32), x[0, 0, 0, :], zt]
                            ).astype(np.float16)
    w_padB = np.concatenate([np.zeros(7, np.float32), x[0, 0, 17, :], zt]
                            ).astype(np.float16)

    conv_w = np.asarray(conv_w)
    E_A = _composite_wav_weights(mu, projA_w, conv_w[0])
    E_B = _composite_wav_weights(mu, projB_w, conv_w[2])
    wts_np = np.zeros((128, 280), np.float16)
    wts_np[:, 0:160] = _eeg_lhsT(conv_w[1])
    wts_np[0:36, 160:280] = _wav_lhsT(E_A)
    wts_np[64:100, 160:280] = _wav_lhsT(E_B)

    in_maps = []
    for k in range(NCORES):
        in_maps.append({
            "eegP": np.ascontiguousarray(_eeg_phases(eeg, k)),
            "wavPA": _wav_phases(w_padA, k),
            "wavPB": _wav_phases(w_padB, k),
            "wts": wts_np,
        })
    return in_maps


def _head(percore, conv_b, fc1_w, fc1_b, fc2_w, fc2_b):
    m = percore.max(axis=0).astype(np.float64)
    eeg_o = m[0:80].reshape(10, 8).max(axis=1)
    wavA_o = m[80:200].reshape(10, 12).max(axis=1)
    wavB_o = m[200:320].reshape(10, 12).max(axis=1)
    conv_b = np.asarray(conv_b, np.float64)
    f = np.concatenate([np.maximum(wavA_o + conv_b[0], 0.0),
                        np.maximum(eeg_o + conv_b[1], 0.0),
                        np.maximum(wavB_o + conv_b[2], 0.0)])
    h = 1.0 / (1.0 + np.exp(-(f @ np.asarray(fc1_w, np.float64).T
                              + np.asarray(fc1_b, np.float64))))
    o = 1.0 / (1.0 + np.exp(-(h @ np.asarray(fc2_w, np.float64).T
                              + np.asarray(fc2_b, np.float64))))
    return o[None, :].astype(np.float32)


def _percore_from_out(arr):
    """Device 'out' [128,10] fp16 -> flat [320] (eeg 80, wavA 120, wavB 120).

    eeg partial maxima in cols 0,1,4,5; wavA in 2,8,9; wavB in 3,6,7."""
    arr = np.asarray(arr, np.float32)
    return np.concatenate([arr[0:80, [0, 1, 4, 5]].max(axis=1),
                           arr[0:120, [2, 8, 9]].max(axis=1),
                           arr[0:120, [3, 6, 7]].max(axis=1)])


def kernel(x, mu, projA_w, projB_w, conv_w, conv_b, fc1_w, fc1_b, fc2_w, fc2_b):
    global LAST_RESULT
    in_maps = _prepare_in_maps(x, mu, projA_w, projB_w, conv_w)
    nc = _get_nc()

    if os.environ.get("KERNEL_USE_SIM"):
        # sim mode for correctness checking without hardware
        from concourse.bass_interp import CoreSim
        percore = np.zeros((NCORES, 320), np.float32)
        for k in range(NCORES):
            sim = CoreSim(nc)
            for name, arr in in_maps[k].items():
                sim.tensor(name)[:] = arr
            sim.simulate()
            percore[k] = _percore_from_out(sim.tensor("out"))
    else:
        from concourse.bass_utils import run_bass_kernel_spmd
        trace = bool(os.environ.get("KERNEL_TRACE"))
        res = run_bass_kernel_spmd(nc, in_maps, list(range(NCORES)),
                                   trace=trace)
        LAST_RESULT = res
        percore = np.stack([_percore_from_out(res.results[k]["out"])
                            for k in range(NCORES)])

    return _head(percore, conv_b, fc1_w, fc1_b, fc2_w, fc2_b)


# revision 16
# speedup vs baseline: 1.2001x; 1.0622x over previous
"""Trainium2 Bass kernel for nn_CNN_88098369175781.

Model: x[1,1,18,T=262144] -> wavA=x[...,0,:], eeg=x[...,1:17,:], wavB=x[...,17,:]
  wav streams: proj(1->16, pointwise) -> diagonal sinc filter bank (15 taps,
  pad 7) -> conv(16->10, 9 taps) + bias -> relu -> global max-pool.
  eeg stream:  conv(16->10, 9 taps) + bias -> relu -> global max-pool.
  concat -> sigmoid FC(30->30) -> sigmoid FC(30->2).

Device decomposition (validated vs reference in numpy):
  * Each wav stream's three linear stages fuse into ONE 1->10 channel, 23-tap
    conv on the zero-padded raw wav signal (weights precomposed on host).
  * Bias/relu commute past the global max (bias is constant over time;
    max(relu(h)) = relu(max(h))), so the device only computes convs + maxima.
  * Convs run on the tensor engine via a polyphase formulation:
      eeg:  time phases r in [0,8), outputs (o, dt in [0,8)) => M=80,
            contraction (c,r) => K=128, 2 accumulating matmuls (u-groups).
      wav:  time phases v in [0,12), outputs (o, dt in [0,12)) => M=120,
            contraction (v,q in [0,3)) => K=36, single matmul per tile
            (the q-replication is materialized host-side).  The A and B
            streams run as row-tiled CONCURRENT matmuls: A-phases live at
            partitions 0:36 (tile_position (0,0)), B-phases at partitions
            64:100 (tile_position (64,0)), halving wav PE time.
  * Operands are fp16 (PSUM accumulation stays fp32): fp32 matmuls run as two
    HW passes on trn2, fp16 single-pass -- and DMA bytes halve.
  * Max-reduction is split across engines: the vector engine reduces eeg PSUM
    directly (fp32); the scalar engine casts wav PSUM to fp16 in SBUF and the
    vector engine reduces those.
  * 8 cores split the time axis (overlapping chunks; overlap is free for max).
  * Host combines per-core maxima and runs the tiny FC head.
"""

import os
import numpy as np

T = 262144
NOUT = T - 8            # 262136 valid conv output positions
NCORES = 8
KLEN = 15
SIGMA = 0.005

EEG_NCOL = 4096         # eeg matmul columns per core (8 outputs each)
EEG_COLS = EEG_NCOL + 1  # phase row length (g=1 needs one extra column)
WAV_NCOL = 2731         # wav matmul columns per core (12 outputs each)
EEG_TC = 8 * EEG_NCOL   # 32768 eeg outputs per core
WAV_TC = 12 * WAV_NCOL  # 32772 wav outputs per core

_NC_CACHE = {}
LAST_RESULT = None      # BassKernelResults of the most recent device run


# --------------------------------------------------------------------------
# host-side weight precompute
# --------------------------------------------------------------------------

def _sinc_rows(mu):
    """Diagonal rows of the reference's sinc_kernel: [16, 15] float64."""
    k = np.linspace(-1.0, 1.0, KLEN)
    kk = (k[None, :] - np.asarray(mu, np.float64)[:, None]) / SIGMA
    nos = np.sum(np.abs(kk) < 1e-5, axis=1)
    kk = np.where((nos >= 0.5)[:, None], kk - 5e-5, kk)
    return np.sin(np.pi * kk) / (np.pi * kk)


def _composite_wav_weights(mu, proj_w, conv_w_i):
    """Fused 1->10ch 23-tap kernel E[o, s] (float64)."""
    krn = _sinc_rows(mu)                                  # [16,15]
    a = np.asarray(proj_w, np.float64)[:, 0, 0]           # [16]
    W = np.asarray(conv_w_i, np.float64)                  # [10,16,9]
    E = np.zeros((10, 23))
    for j in range(9):
        E[:, j:j + 15] += np.einsum('oc,cm->om', W[:, :, j] * a[None, :], krn)
    return E


def _eeg_lhsT(W1):
    """[128, 160]: cols g*80+(o*8+dt); row c*8+r; val W1[o,c,8g+r-dt]."""
    W1 = np.asarray(W1, np.float64)
    out = np.zeros((128, 160))
    g, c, r, o, dt = np.meshgrid(np.arange(2), np.arange(16), np.arange(8),
                                 np.arange(10), np.arange(8), indexing='ij')
    j = 8 * g + r - dt
    valid = (j >= 0) & (j < 9)
    out[(c * 8 + r)[valid], (g * 80 + o * 8 + dt)[valid]] = \
        W1[o[valid], c[valid], np.clip(j[valid], 0, 8)]
    return out.astype(np.float32)


def _wav_lhsT(E):
    """[36, 120]: row v*3+q, col o*12+dt, val E[o, 12q+v-dt]."""
    out = np.zeros((36, 120))
    v, q, o, dt = np.meshgrid(np.arange(12), np.arange(3), np.arange(10),
                              np.arange(12), indexing='ij')
    s = 12 * q + v - dt
    valid = (s >= 0) & (s < 23)
    out[(v * 3 + q)[valid], (o * 12 + dt)[valid]] = E[o[valid], np.clip(s[valid], 0, 22)]
    return out.astype(np.float32)


# --------------------------------------------------------------------------
# host-side per-core input slicing
# --------------------------------------------------------------------------

def _core_starts(k):
    return (min(k * 32767, NOUT - EEG_TC), min(k * 32767, NOUT - WAV_TC))


def _eeg_phases(eeg, k):
    """[128, 4097]: row c*8+r, col m = eeg[c, s_e + 8m + r]."""
    s_e, _ = _core_starts(k)
    v = eeg[:, s_e:s_e + 8 * EEG_COLS]                  # [16, 32776]
    p = v.reshape(16, EEG_COLS, 8).transpose(0, 2, 1)   # [16,8,4097]
    return p.reshape(128, EEG_COLS)


def _wav_phases(w_pad, k):
    """[36, 2731]: row v*3+q, col n = w_pad[s_w + 12(n+q) + v]."""
    _, s_w = _core_starts(k)
    sl = w_pad[s_w:s_w + 12 * (WAV_NCOL + 2)]
    y = sl.reshape(WAV_NCOL + 2, 12).T                  # y[v,m] = sl[12m+v]
    out = np.empty((36, WAV_NCOL), dtype=w_pad.dtype)
    for q in range(3):
        out[q::3, :] = y[:, q:q + WAV_NCOL]
    return np.ascontiguousarray(out)


# --------------------------------------------------------------------------
# bass kernel
# --------------------------------------------------------------------------

def _build_nc():
    import concourse.bacc as bacc
    import concourse.tile as tile
    import concourse.mybir as mybir

    f32 = mybir.dt.float32
    f16 = mybir.dt.float16
    nc = bacc.Bacc("TRN2", target_bir_lowering=False, debug=False,
                   num_devices=NCORES)

    eegP = nc.dram_tensor("eegP", [128, EEG_COLS], f16, kind="ExternalInput")
    wavP = nc.dram_tensor("wavP", [36, 2 * WAV_NCOL], f16, kind="ExternalInput")
    wts = nc.dram_tensor("wts", [128, 400], f16, kind="ExternalInput")
    out = nc.dram_tensor("out", [128, 10], f16, kind="ExternalOutput")

    N_ECHUNK = 2                 # eeg input loaded in 2 column chunks
    ECHUNK = 2048                # chunk j covers cols [2048j, 2048j+2049)
    N_WARM = 6                   # dummy matmuls to warm the PE clock gate

    with tile.TileContext(nc) as tc:
        with (
            tc.tile_pool(name="sb", bufs=1) as sb,
            tc.tile_pool(name="ps", bufs=4, space="PSUM") as psp,
        ):
            # PE warmup: dummy matmuls on a zeroed scratch tile keep the PE
            # busy while the first input DMAs land, so the HAM clock-gate
            # opens (1.2 -> 2.4 GHz) before the real matmuls start.
            scr = sb.tile([128, 512], f16, tag="scr")
            nc.gpsimd.memset(scr[:], 0.0)
            wps = psp.tile([120, 1024], f32, tag="ps", name="wps")
            for _ in range(N_WARM):
                nc.tensor.matmul(wps[0:80, 0:512], scr[:, 0:80], scr[:],
                                 start=True, stop=True)

            # input DMAs spread over three issue engines: descriptor
            # generation costs ~0.7us per dma_start and serializes per engine
            echunks = [sb.tile([128, ECHUNK + 1], f16, tag=f"eegchunk{j}",
                               name=f"eegchunk{j}") for j in range(N_ECHUNK)]
            wts_t = sb.tile([128, 400], f16, tag="wts")
            wav_t = sb.tile([36, 2 * WAV_NCOL], f16, tag="wav")
            nc.scalar.dma_start(wts_t[:], wts[:])
            nc.sync.dma_start(echunks[0][:], eegP[:, 0:ECHUNK + 1])
            nc.sync.dma_start(echunks[1][:], eegP[:, ECHUNK:2 * ECHUNK + 1])
            nc.scalar.dma_start(wav_t[:], wavP[:])
            wE_t = wts_t[:, 0:160]

            # merged fp16 output tile; host finishes the max over columns
            # cols: 0=eeg p0, 1=eeg p2, 2=wavA tail, 3=wavB tail (fp32 path),
            #       4=eeg p1, 5=eeg p3, 6,7=wavB p0,p1, 8,9=wavA p0,p1 (fp16)
            out16 = sb.tile([128, 10], f16, tag="out16")
            nc.gpsimd.memset(out16[:], 0.0)
            # fp32 maxima landing tile for the PSUM-direct reduces
            mF = sb.tile([120, 4], f32, tag="mF")
            nc.gpsimd.memset(mF[:], 0.0)
            # fp16 staging tiles for the ACT-evacuated psum pairs
            stg = [sb.tile([120, 1024], f16, tag=f"stg{i}", name=f"stg{i}")
                   for i in range(4)]

            X = mybir.AxisListType.X
            Copy = mybir.ActivationFunctionType.Copy

            # PSUM evacuation is split: ACT casts some banks to fp16 SBUF
            # ((N+352)/1.2 per op) while DVE direct-reduces the others from
            # PSUM (1 elem/cycle), then reduces the fp16 staged tiles.

            # eeg: pairs 0,2 -> DVE direct; pairs 1,3 -> ACT cast
            for p in range(4):
                ch = echunks[p // 2]
                base = (p % 2) * 1024
                ps = psp.tile([120, 1024], f32, tag="ps", name=f"pse{p}")
                for g in range(2):
                    for j in range(2):
                        lo = j * 512
                        nc.tensor.matmul(ps[0:80, lo:lo + 512],
                                         wE_t[:, 80 * g:80 * g + 80],
                                         ch[:, base + lo + g:base + lo + g + 512],
                                         start=(g == 0), stop=(g == 1))
                if p % 2 == 0:
                    nc.vector.reduce_max(mF[0:80, p // 2:p // 2 + 1],
                                         ps[0:80, :], axis=X)
                else:
                    nc.scalar.activation(stg[p // 2][0:80, :], ps[0:80, :], Copy)
            nc.vector.reduce_max(out16[0:80, 4:5], stg[0][0:80, :], axis=X)
            nc.vector.reduce_max(out16[0:80, 5:6], stg[1][0:80, :], axis=X)

            # wav: per stream, pairs 0,1 -> ACT cast + fp16 reduce; tail (683
            # cols) -> DVE direct
            for si in range(2):
                s0, s1 = (stg[2], stg[3]) if si == 0 else (stg[0], stg[1])
                for p in range(3):
                    ps = psp.tile([120, 1024], f32, tag="ps", name=f"psw{si}{p}")
                    for j in range(2):
                        n0 = si * WAV_NCOL + (2 * p + j) * 512
                        nn = min(512, (si + 1) * WAV_NCOL - n0)
                        nc.tensor.matmul(ps[:, j * 512:j * 512 + nn],
                                         wts_t[0:36, 160 + 120 * si:280 + 120 * si],
                                         wav_t[:, n0:n0 + nn],
                                         start=True, stop=True)
                    if p < 2:
                        nc.scalar.activation((s0 if p == 0 else s1)[:], ps[:], Copy)
                    else:
                        nc.vector.reduce_max(mF[:, 2 + si:3 + si],
                                             ps[:, 0:683], axis=X)
                nc.vector.reduce_max(out16[0:120, 6 + 2 * si:7 + 2 * si],
                                     s0[:], axis=X)
                nc.vector.reduce_max(out16[0:120, 7 + 2 * si:8 + 2 * si],
                                     s1[:], axis=X)

            # cast the fp32 maxima into the merged fp16 output tile
            nc.vector.tensor_copy(out16[0:120, 0:4], mF[:])

            nc.sync.dma_start(out[:], out16[:])

    nc.compile()
    return nc


def _get_nc():
    if "nc" not in _NC_CACHE:
        _NC_CACHE["nc"] = _build_nc()
    return _NC_CACHE["nc"]


# --------------------------------------------------------------------------
# entry point
# --------------------------------------------------------------------------

def _prepare_in_maps(x, mu, projA_w, projB_w, conv_w):
    x = np.asarray(x, np.float32)
    eeg = np.ascontiguousarray(x[0, 0, 1:17, :]).astype(np.float16)
    zt = np.zeros(64, np.float32)
    w_padA = np.concatenate([np.zeros(7, np.float32), x[0, 0, 0, :], zt]
                            ).astype(np.float16)
    w_padB = np.concatenate([np.zeros(7, np.float32), x[0, 0, 17, :], zt]
                            ).astype(np.float16)

    conv_w = np.asarray(conv_w)
    E_A = _composite_wav_weights(mu, projA_w, conv_w[0])
    E_B = _composite_wav_weights(mu, projB_w, conv_w[2])
    wts_np = np.zeros((128, 400), np.float16)
    wts_np[:, 0:160] = _eeg_lhsT(conv_w[1])
    wts_np[0:36, 160:280] = _wav_lhsT(E_A)
    wts_np[0:36, 280:400] = _wav_lhsT(E_B)

    in_maps = []
    for k in range(NCORES):
        wavp = np.concatenate([_wav_phases(w_padA, k), _wav_phases(w_padB, k)],
                              axis=1)
        in_maps.append({
            "eegP": np.ascontiguousarray(_eeg_phases(eeg, k)),
            "wavP": np.ascontiguousarray(wavp),
            "wts": wts_np,
        })
    return in_maps


def _head(percore, conv_b, fc1_w, fc1_b, fc2_w, fc2_b):
    m = percore.max(axis=0).astype(np.float64)
    eeg_o = m[0:80].reshape(10, 8).max(axis=1)
    wavA_o = m[80:200].reshape(10, 12).max(axis=1)
    wavB_o = m[200:320].reshape(10, 12).max(axis=1)
    conv_b = np.asarray(conv_b, np.float64)
    f = np.concatenate([np.maximum(wavA_o + conv_b[0], 0.0),
                        np.maximum(eeg_o + conv_b[1], 0.0),
                        np.maximum(wavB_o + conv_b[2], 0.0)])
    h = 1.0 / (1.0 + np.exp(-(f @ np.asarray(fc1_w, np.float64).T
                              + np.asarray(fc1_b, np.float64))))
    o = 1.0 / (1.0 + np.exp(-(h @ np.asarray(fc2_w, np.float64).T
                              + np.asarray(fc2_b, np.float64))))
    return o[None, :].astype(np.float32)


def _percore_from_out(arr):
    """Device 'out' [128,10] fp16 -> flat [320] (eeg 80, wavA 120, wavB 120).

    eeg partial maxima in cols 0,1,4,5; wavA in 2,6,7; wavB in 3,8,9."""
    arr = np.asarray(arr, np.float32)
    return np.concatenate([arr[0:80, [0, 1, 4, 5]].max(axis=1),
                           arr[0:120, [2, 6, 7]].max(axis=1),
                           arr[0:120, [3, 8, 9]].max(axis=1)])


def kernel(x, mu, projA_w, projB_w, conv_w, conv_b, fc1_w, fc1_b, fc2_w, fc2_b):
    global LAST_RESULT
    in_maps = _prepare_in_maps(x, mu, projA_w, projB_w, conv_w)
    nc = _get_nc()

    if os.environ.get("KERNEL_USE_SIM"):
        # sim mode for correctness checking without hardware
        from concourse.bass_interp import CoreSim
        percore = np.zeros((NCORES, 320), np.float32)
        for k in range(NCORES):
            sim = CoreSim(nc)
            for name, arr in in_maps[k].items():
                sim.tensor(name)[:] = arr
            sim.simulate()
            percore[k] = _percore_from_out(sim.tensor("out"))
    else:
        from concourse.bass_utils import run_bass_kernel_spmd
        trace = bool(os.environ.get("KERNEL_TRACE"))
        res = run_bass_kernel_spmd(nc, in_maps, list(range(NCORES)),
                                   trace=trace)
        LAST_RESULT = res
        percore = np.stack([_percore_from_out(res.results[k]["out"])
                            for k in range(NCORES)])

    return _head(percore, conv_b, fc1_w, fc1_b, fc2_w, fc2_b)
